# revision 1
# baseline (speedup 1.0000x reference)
"""Trainium2 Bass kernel: single-head attention module (dense transformer).

Computes, for x [4, 4096, 256] (f32) and per-projection weights/biases:
    q = x @ Wq + bq;  k = x @ Wk + bk;  v = x @ Wv + bv
    out = softmax((q k^T) / sqrt(256)) @ v @ Wo + bo

Sharding over 8 NeuronCores: core c handles batch c//2, query half c%2.
The host rotates each core's batch so its queries are always rows 0..2047
(softmax is key-order invariant), keeping the device program identical
across cores. Each core computes K/V for its whole batch (redundant with
its pair core, which is cheap) and attention + output projection for its
2048 queries.

Per-core kernel layout (matmuls in float32r = full-rate ~fp32; every
matmul operand tile is declared float32r so its producer rounds on write,
which the BIR verifier requires):
  - x is loaded in natural [s,d] tiles (1 MiB DMAs — each dma_start costs
    ~650 ns on both the issuing sequencer and the shared HWDGE) and
    transposed on the PE (via identity) to x^T [d, s] so projections can
    contract over d on the partition axis.
  - Q^T [e, sq] and K^T [e, sk] are produced directly transposed
    (lhsT = W chunk, moving = x^T), which is the exact layout the scores
    matmul wants: S^T[sk_tile, sq] = (K^T chunk).T @ Q^T chunk.
  - Softmax over keys is computed WITHOUT max subtraction (scores here
    are bounded by ~±10, and softmax-no-max is the same function): P^T =
    exp(S^T/16) on the scalar engine straight out of PSUM.
  - The PV product accumulates out^T[e, sq] over the 32 key tiles in
    PSUM. The softmax denominator comes from a ones[128,128] stationary
    matmul over DVE-computed sums of four P^T tiles (the quad-sum quarters
    the extra PE stream), accumulated broadcast across all partitions.
  - out^T is scaled by 1/denom (DVE) and fed as the stationary operand of
    the final projection, which lands the output in natural [sq, f]
    layout for contiguous paired 256-row output DMAs.

Measured: rel err 2.9e-04 vs the fp32 reference on TRN2 (f32r rounding,
matches a TF32-emulation estimate). Cost-model exec 169 us/core: ~143 us
TensorE busy (87% saturated; scores 55 + PV 55 + denom 7 + projections 17
+ transposes 10), ~97 us ACT (exp), ~91 us DVE. Remaining non-PE time is
startup DMA (~3.5 us), the fixed end-of-kernel drain barrier (~4 us), the
last block's reciprocal chain (~3 us), and scattered sub-200 ns semaphore
latencies. Next levers if iterating further (needs a real neuron-profile
trace): verify f32r matmuls hit 1 cycle/row on HW back-to-back, and
whether the scores->exp->PV chain holds PE saturation under real ACT
latencies.
"""

import numpy as np

import concourse.bass as bass  # noqa: F401  (AP types come through tile/bacc)
import concourse.tile as tile
from concourse import bacc, mybir
from concourse.bass_utils import run_bass_kernel_spmd
from concourse.masks import make_identity

B, S, D = 4, 4096, 256
SQ = S // 2  # queries per core
NCORES = 8
F32 = mybir.dt.float32
F32R = mybir.dt.float32r
SCALE = 1.0 / 16.0  # 1/sqrt(D)


def _r(ap):
    """View an fp32 AP as float32r: full-rate fp32 matmul on the PE."""
    return ap.bitcast(F32R)


def _build(phases=3):
    nc = bacc.Bacc("TRN2", target_bir_lowering=False, debug=False,
                   num_devices=NCORES)

    xkv = nc.dram_tensor("xkv", [S, D], F32, kind="ExternalInput").ap()
    w_dram = {
        n: nc.dram_tensor(n, [D, D], F32, kind="ExternalInput").ap()
        for n in ("wq", "wk", "wv", "wo")
    }
    b_dram = {
        n: nc.dram_tensor(n, [D], F32, kind="ExternalInput").ap()
        for n in ("bq", "bk", "bo")
    }
    out = nc.dram_tensor("out", [SQ, D], F32, kind="ExternalOutput").ap()

    bq_col = b_dram["bq"].rearrange("(a b) -> a b", b=1)  # [256, 1]
    bk_col = b_dram["bk"].rearrange("(a b) -> a b", b=1)
    bo_row = b_dram["bo"].rearrange("(a b) -> a b", a=1)  # [1, 256]
    # Grouped views for wide DMAs: one instruction per ~1 MiB, since each
    # dma_start costs ~650 ns on the issuing sequencer AND on the shared HWDGE.
    xkv_g = xkv.rearrange("(g j p) c -> g p j c", j=8, p=128)   # [4,128,8,256]
    w_g = {n: w.rearrange("(j p) c -> p j c", j=2) for n, w in w_dram.items()}
    out_g = out.rearrange("(g j p) c -> g p j c", j=2, p=128)   # [8,128,2,256]

    with tile.TileContext(nc) as tc:
        with (
            tc.tile_pool(name="const", bufs=1) as cpool,
            tc.tile_pool(name="xin", bufs=4) as xin_pool,
            tc.tile_pool(name="pt", bufs=4) as pt_pool,
            tc.tile_pool(name="ovec", bufs=2) as ovec_pool,
            tc.tile_pool(name="fout", bufs=2) as fout_pool,
            tc.tile_pool(name="psmm", bufs=1, space="PSUM") as psmm,
            tc.tile_pool(name="psacc", bufs=1, space="PSUM") as psacc,
        ):
            # ---- constants ----
            ident = cpool.tile([128, 128], F32, tag="ident", name="ident")
            make_identity(nc, ident[:])
            ident_r = cpool.tile([128, 128], F32R, tag="identr", name="identr")
            nc.vector.tensor_copy(ident_r[:], ident[:])
            ones128 = cpool.tile([128, 128], F32R, tag="ones128", name="ones128")
            # memset can't target f32r; write the 1.0f bit pattern via uint32
            nc.vector.memset(ones128[:].bitcast(mybir.dt.uint32), 0x3F800000)
            ones1 = cpool.tile([1, 128], F32, tag="ones1", name="ones1")
            nc.vector.memset(ones1[:], 1.0)

            # ---- x DMAs first: everything depends on x, so it must win the
            # HWDGE queue ahead of the constant loads. Group 0 is split so the
            # first transposes can start after ~0.25 MiB.
            xt_tiles = []
            for g in range(4):
                xt = xin_pool.tile([128, 8 * D], F32R, tag="xin", name="xin")
                xt_j = xt.rearrange("p (j c) -> p j c", j=8)
                if g == 0:
                    nc.sync.dma_start(xt_j[:, 0:2], _r(xkv_g[g][:, 0:2]))
                    nc.sync.dma_start(xt_j[:, 2:8], _r(xkv_g[g][:, 2:8]))
                else:
                    nc.sync.dma_start(xt_j, _r(xkv_g[g]))
                xt_tiles.append(xt)

            w_sb = {}
            for n in ("wq", "wk", "wv", "wo"):
                t = cpool.tile([128, 2 * D], F32R, tag=f"w_{n}", name=f"w_{n}")
                nc.sync.dma_start(
                    t.rearrange("p (j c) -> p j c", j=2), _r(w_g[n][:]))
                w_sb[n] = t

            def wchunk(n, c):  # [128, 256] d-chunk c of W
                return w_sb[n][:, c * D:(c + 1) * D]

            bqc, bkc = [], []
            for c in range(2):
                t = cpool.tile([128, 1], F32, tag=f"bq{c}", name=f"bq{c}")
                nc.sync.dma_start(t[:], bq_col[c * 128:(c + 1) * 128, :])
                bqc.append(t)
                t = cpool.tile([128, 1], F32, tag=f"bk{c}", name=f"bk{c}")
                nc.sync.dma_start(t[:], bk_col[c * 128:(c + 1) * 128, :])
                bkc.append(t)

            # bo broadcast across partitions: ones1[1,128].T @ bo_row[1,256],
            # then duplicated side by side so one [128,512] add covers two
            # output row-tiles. (bv is folded into bo host-side: attention
            # rows sum to 1, so attn@(v+bv)@Wo + bo == attn@v@Wo + (bv@Wo+bo).)
            bob = cpool.tile([128, 2 * D], F32, tag="bob", name="bob")
            row = cpool.tile([1, D], F32, tag="bor", name="bor")
            nc.sync.dma_start(row[:], bo_row[:])
            bps = psmm.tile([128, D], F32, tag="fp", name="fp", bufs=1)
            nc.tensor.matmul(bps[:], ones1[:], row[:], start=True, stop=True)
            nc.vector.tensor_copy(bob[:, 0:D], bps[:])
            nc.vector.tensor_copy(bob[:, D:2 * D], bps[:])

            # ---- persistent activations ----
            xkvT = [cpool.tile([128, S], F32R, tag=f"xkvT{c}", name=f"xkvT{c}")
                    for c in range(2)]
            qT = [cpool.tile([128, SQ], F32R, tag=f"qT{c}", name=f"qT{c}")
                  for c in range(2)]
            kT = [cpool.tile([128, S], F32R, tag=f"kT{c}", name=f"kT{c}")
                  for c in range(2)]
            v_sb = cpool.tile([128, 32 * D], F32R, tag="v", name="v")

            # ---- phase 1: load x (1 MiB DMAs), transpose to x^T ----
            # Four 128x128 transposes land in one [128,512] PSUM bank; the
            # single wide eviction alternates between DVE and ACT so neither
            # engine becomes the phase bottleneck.
            evict_parity = 0
            for dst, ngrp in ((xkvT, 4),):
                for g in range(ngrp):
                    xt = xt_tiles[g]
                    for half in range(2):
                        for c in range(2):
                            tp = psmm.tile([128, 512], F32, tag="sc",
                                           name="sc", bufs=4)
                            for j in range(4):
                                jj = half * 4 + j
                                nc.tensor.transpose(
                                    _r(tp[:, j * 128:(j + 1) * 128]),
                                    xt[:, jj * D + c * 128:
                                       jj * D + (c + 1) * 128],
                                    ident_r[:])
                            col0 = (g * 8 + half * 4) * 128
                            dsl = dst[c][:, col0:col0 + 512]
                            if evict_parity % 2 == 0:
                                nc.vector.tensor_copy(dsl, tp[:])
                            else:
                                nc.scalar.copy(dsl, tp[:])
                            evict_parity += 1

            # ---- phase 2: projections ----
            # Q^T / K^T: lhsT = W[d_chunk, e_tile], moving = x^T[d_chunk, s]
            for (wn, xT, dstT, bcol, stot) in () if phases < 2 else (
                ("wq", xkvT, qT, bqc, SQ),
                ("wk", xkvT, kT, bkc, S),
            ):
                for et in range(2):
                    for blk in range(stot // 512):
                        pp = psmm.tile([128, 512], F32, tag="sc", name="sc",
                                       bufs=4)
                        for c in range(2):
                            nc.tensor.matmul(
                                pp[:],
                                _r(wchunk(wn, c)[:, et * 128:(et + 1) * 128]),
                                _r(xT[c][:, blk * 512:(blk + 1) * 512]),
                                start=(c == 0), stop=(c == 1),
                            )
                        dsl = dstT[et][:, blk * 512:(blk + 1) * 512]
                        if evict_parity % 2 == 0:
                            nc.vector.tensor_scalar_add(dsl, pp[:], bcol[et][:])
                        else:
                            nc.scalar.activation(
                                dsl, pp[:],
                                mybir.ActivationFunctionType.Identity,
                                bias=bcol[et][:])
                        evict_parity += 1

            # V: natural layout [sk, e]; lhsT = x^T[d_chunk, sk_tile].
            # Two sk-tiles share one [128,512] PSUM bank -> one wide eviction.
            for stp in range(16 if phases >= 2 else 0):
                vp = psmm.tile([128, 512], F32, tag="sc", name="sc", bufs=4)
                for half in range(2):
                    st = stp * 2 + half
                    for c in range(2):
                        nc.tensor.matmul(
                            vp[:, half * D:(half + 1) * D],
                            _r(xkvT[c][:, st * 128:(st + 1) * 128]),
                            _r(wchunk("wv", c)),
                            start=(c == 0), stop=(c == 1),
                        )
                dsl = v_sb[:, stp * 512:(stp + 1) * 512]
                if evict_parity % 2 == 0:
                    nc.vector.tensor_copy(dsl, vp[:])
                else:
                    nc.scalar.copy(dsl, vp[:])
                evict_parity += 1

            # ---- phase 3: attention ----
            for qb in range(SQ // 512 if phases >= 3 else 0):
                qsl = slice(qb * 512, (qb + 1) * 512)
                acc = [psacc.tile([128, 512], F32, tag=f"acc{e}",
                                  name=f"acc{e}") for e in range(2)]
                accd = psacc.tile([128, 512], F32, tag="accd", name="accd")
                ptq = []
                for st in range(32):
                    ssl = slice(st * 128, (st + 1) * 128)
                    sp = psmm.tile([128, 512], F32, tag="sc", name="sc",
                                   bufs=4)
                    nc.tensor.matmul(sp[:], _r(kT[0][:, ssl]),
                                     _r(qT[0][:, qsl]), start=True, stop=False)
                    nc.tensor.matmul(sp[:], _r(kT[1][:, ssl]),
                                     _r(qT[1][:, qsl]), start=False, stop=True)
                    pt = pt_pool.tile([128, 512], F32R, tag="pt", name="pt", bufs=6)
                    nc.scalar.activation(pt[:], sp[:],
                                         mybir.ActivationFunctionType.Exp,
                                         scale=SCALE)
                    first, last = (st == 0), (st == 31)
                    nc.tensor.matmul(acc[0][:], _r(v_sb[:, st * D:st * D + 128]),
                                     _r(pt[:]), start=first, stop=last)
                    nc.tensor.matmul(acc[1][:],
                                     _r(v_sb[:, st * D + 128:(st + 1) * D]),
                                     _r(pt[:]), start=first, stop=last)
                    # Denominator: sum pt quads on DVE (off the PE's
                    # critical path), quartering the ones-matmul streams.
                    ptq.append(pt)
                    if st % 4 == 3:
                        pa = pt_pool.tile([128, 512], F32R, tag="ptsum",
                                          name="ptsum")
                        nc.vector.tensor_add(pa[:], ptq[0][:], ptq[1][:])
                        pb = pt_pool.tile([128, 512], F32R, tag="ptsum",
                                          name="ptsum")
                        nc.vector.tensor_add(pb[:], ptq[2][:], ptq[3][:])
                        pc = pt_pool.tile([128, 512], F32R, tag="ptsum",
                                          name="ptsum")
                        nc.vector.tensor_add(pc[:], pa[:], pb[:])
                        nc.tensor.matmul(accd[:], _r(ones128[:]), _r(pc[:]),
                                         start=(st == 3), stop=(st == 31))
                        ptq = []

                rec = ovec_pool.tile([128, 512], F32, tag="rec", name="rec")
                o = [ovec_pool.tile([128, 512], F32R, tag=f"o{e}",
                                    name=f"o{e}") for e in range(2)]
                # halves: lets the first final matmuls start ~0.8us earlier
                for hsl in (slice(0, 256), slice(256, 512)):
                    nc.vector.reciprocal(rec[:, hsl], accd[:, hsl])
                    for e in range(2):
                        nc.vector.tensor_mul(o[e][:, hsl], acc[e][:, hsl],
                                             rec[:, hsl])

                # Final projection: two row-tiles per [128,512] staging tile,
                # one paired 256-row output DMA.
                for pair in range(2):
                    fo = fout_pool.tile([128, 2 * D], F32, tag="fout",
                                        name="fout")
                    for half in range(2):
                        t4 = pair * 2 + half
                        tsl = slice(t4 * 128, (t4 + 1) * 128)
                        fp = psmm.tile([128, D], F32, tag="fp", name="fp",
                                       bufs=1)
                        for e in range(2):
                            nc.tensor.matmul(fp[:], _r(o[e][:, tsl]),
                                             _r(wchunk("wo", e)),
                                             start=(e == 0), stop=(e == 1))
                        nc.vector.tensor_add(fo[:, half * D:(half + 1) * D],
                                             fp[:], bob[:, 0:D])
                    nc.sync.dma_start(out_g[qb * 2 + pair],
                                      fo.rearrange("p (j c) -> p j c", j=2))

    nc.compile()
    return nc



_NC = None


def _get_nc():
    global _NC
    if _NC is None:
        _NC = _build()
    return _NC


class _Runner:
    """Cached jitted SPMD executor (run_bass_kernel_spmd rebuilds its jax
    closure every call, forcing a retrace; this traces once)."""

    def __init__(self, nc):
        import jax
        from jax.sharding import Mesh, PartitionSpec
        from jax.experimental.shard_map import shard_map
        from concourse import bass2jax, mybir as mb

        bass2jax.install_neuronx_cc_hook()
        self.jax = jax
        if not any("axon" in str(getattr(d, "platform", "")).lower()
                   or str(d).startswith("NC_")
                   for d in jax.devices()):
            # jax was initialized on another platform (e.g. cpu for the
            # reference); reset so the axon NeuronCores are visible.
            import jax._src.xla_bridge as xb
            jax.config.update("jax_platforms", None)
            xb._clear_backends()
            if hasattr(xb.get_backend, "cache_clear"):
                xb.get_backend.cache_clear()
            if not any("axon" in str(getattr(d, "platform", "")).lower()
                       or str(d).startswith("NC_")
                       for d in jax.devices()):
                jax.config.update("jax_platforms", "axon")
                xb._clear_backends()
                if hasattr(xb.get_backend, "cache_clear"):
                    xb.get_backend.cache_clear()
        partition_name = (nc.partition_id_tensor.name
                          if nc.partition_id_tensor else None)
        in_names, out_names, out_avals = [], [], []
        for alloc in nc.m.functions[0].allocations:
            if not isinstance(alloc, mb.MemoryLocationSet):
                continue
            name = alloc.memorylocations[0].name
            if alloc.kind == "ExternalInput":
                if name != partition_name:
                    in_names.append(name)
            elif alloc.kind == "ExternalOutput":
                out_names.append(name)
                out_avals.append(jax.core.ShapedArray(
                    tuple(alloc.tensor_shape), mb.dt.np(alloc.dtype)))
        self.in_names, self.out_names, self.out_avals = \
            in_names, out_names, out_avals
        n_params, n_outs = len(in_names), len(out_names)
        bind_in_names = in_names + out_names + (
            [partition_name] if partition_name else [])

        def _body(*args):
            operands = list(args)
            if partition_name is not None:
                operands.append(bass2jax.partition_id_tensor())
            outs = bass2jax._bass_exec_p.bind(
                *operands,
                out_avals=tuple(out_avals),
                in_names=tuple(bind_in_names),
                out_names=tuple(out_names),
                lowering_input_output_aliases=(),
                sim_require_finite=True,
                sim_require_nnan=True,
                nc=nc,
            )
            return tuple(outs)

        devices = jax.devices()[:NCORES]
        mesh = Mesh(np.asarray(devices), ("core",))
        spec = (PartitionSpec("core"),) * (n_params + n_outs)
        self.fn = jax.jit(
            shard_map(_body, mesh=mesh, in_specs=spec,
                      out_specs=(PartitionSpec("core"),) * n_outs,
                      check_rep=False),
            donate_argnums=tuple(range(n_params, n_params + n_outs)),
            keep_unused=True,
        )

    def run(self, in_maps):
        concat_in = [
            np.concatenate([np.asarray(m[n]) for m in in_maps], axis=0)
            for n in self.in_names
        ]
        concat_zeros = [
            np.zeros((NCORES * a.shape[0], *a.shape[1:]), a.dtype)
            for a in self.out_avals
        ]
        outs = self.fn(*concat_in, *concat_zeros)
        return [
            {n: np.asarray(outs[i]).reshape(NCORES, *self.out_avals[i].shape)[c]
             for i, n in enumerate(self.out_names)}
            for c in range(NCORES)
        ]


_RUNNER = None


def _get_runner():
    global _RUNNER
    if _RUNNER is None:
        _RUNNER = _Runner(_get_nc())
    return _RUNNER


def kernel(**inputs):
    x = np.ascontiguousarray(np.asarray(inputs["x"], dtype=np.float32))
    Wq = np.ascontiguousarray(np.asarray(inputs["Wq"], dtype=np.float32))
    Wk = np.ascontiguousarray(np.asarray(inputs["Wk"], dtype=np.float32))
    Wv = np.ascontiguousarray(np.asarray(inputs["Wv"], dtype=np.float32))
    Wo = np.ascontiguousarray(np.asarray(inputs["Wo"], dtype=np.float32))
    bq = np.ascontiguousarray(np.asarray(inputs["bq"], dtype=np.float32))
    bk = np.ascontiguousarray(np.asarray(inputs["bk"], dtype=np.float32))
    bv = np.ascontiguousarray(np.asarray(inputs["bv"], dtype=np.float32))
    bo = np.ascontiguousarray(np.asarray(inputs["bo"], dtype=np.float32))

    try:
        runner = _get_runner()
    except Exception:
        runner = None
    # bv folds into bo: attention rows sum to 1, so attn@(v+bv) = attn@v + bv.
    bo_eff = (bv @ Wo + bo).astype(np.float32)
    in_maps = []
    for c in range(NCORES):
        b, h = divmod(c, 2)
        # Rotate the batch so this core's queries are rows 0..SQ-1; keys and
        # values see all rows either way (softmax is key-order invariant).
        xb = x[b] if h == 0 else np.ascontiguousarray(
            np.concatenate([x[b, SQ:], x[b, :SQ]]))
        in_maps.append({
            "xkv": xb,
            "wq": Wq, "wk": Wk, "wv": Wv, "wo": Wo,
            "bq": bq, "bk": bk, "bo": bo_eff,
        })
    results = None
    if runner is not None:
        try:
            results = runner.run(in_maps)
        except Exception:
            results = None
    if results is None:
        results = run_bass_kernel_spmd(
            _get_nc(), in_maps, core_ids=list(range(NCORES))).results
    outp = np.empty((B, S, D), dtype=np.float32)
    for c in range(NCORES):
        b, h = divmod(c, 2)
        outp[b, h * SQ:(h + 1) * SQ] = results[c]["out"]
    return outp



# revision 45
# speedup vs baseline: 1.7283x; 1.7283x over previous
"""Trainium2 Bass kernel: single-head attention module (dense transformer).

Computes, for x [4, 4096, 256] (f32) and per-projection weights/biases:
    q = x @ Wq + bq;  k = x @ Wk + bk;  v = x @ Wv + bv
    out = softmax((q k^T) / sqrt(256)) @ v @ Wo + bo

Sharding over 8 NeuronCores: core c handles batch c//2, query half c%2.
The host rotates each core's batch so its queries are always rows 0..2047.

Algebraic restructure vs the straightforward kernel (weight-weight products
are precomputed host-side; they are 256^3 and exact):
  - scores = x_q (Wq Wk^T) x_k^T + per-key bias d, with M = Wq Wk^T and
    d = x_k (Wk bq) (per-query terms and constants are softmax-invariant).
    This removes the K projection entirely; x^T serves as the key operand.
  - out = (P x_k) (Wv Wo) / denom + (bv Wo + bo): reassociating P V Wo as
    (P x) Wvo removes the V projection; natural-layout x tiles are the
    stationary operand of the PV matmul, Wvo = Wv Wo folds the two output
    projections into one.
Device work per core: 64 transposes (x^T), G = M^T x_q^T (8k cycles),
scores (131k), PV (131k), softmax denominator via an eager DVE add-chain +
one ones-matmul per 512-query block (2k), final projection (8k).

Emission is software-pipelined: scores/exp run one step ahead of PV; the
g1..g3 transposes and the G blocks for later query-blocks are spread into
early slots of the first two query-block loops; each block's denominator /
reciprocal / final projection is interleaved into the next block's first
four slots so the PE never drains.
"""

import numpy as np

import concourse.bass as bass  # noqa: F401
import concourse.tile as tile
from concourse import bacc, mybir
from concourse.bass_utils import run_bass_kernel_spmd
from concourse.masks import make_identity

B, S, D = 4, 4096, 256
SQ = S // 2  # queries per core
NCORES = 8
F32 = mybir.dt.float32
F32R = mybir.dt.float32r
SCALE = 1.0 / 16.0  # 1/sqrt(D)
EXP = mybir.ActivationFunctionType.Exp


def _r(ap):
    """View an fp32 AP as float32r: full-rate fp32 matmul on the PE."""
    return ap.bitcast(F32R)


def _build():
    nc = bacc.Bacc("TRN2", target_bir_lowering=False, debug=False,
                   num_devices=NCORES)

    xkv = nc.dram_tensor("xkv", [S, D], F32, kind="ExternalInput").ap()
    m_dram = nc.dram_tensor("mqk", [D, D], F32, kind="ExternalInput").ap()
    wvo_dram = nc.dram_tensor("wvo", [D, D], F32, kind="ExternalInput").ap()
    dpos_dram = nc.dram_tensor("dpos", [128, 32], F32,
                               kind="ExternalInput").ap()
    bo_dram = nc.dram_tensor("bo", [D], F32, kind="ExternalInput").ap()
    out = nc.dram_tensor("out", [SQ, D], F32, kind="ExternalOutput").ap()

    bo_row = bo_dram.rearrange("(a b) -> a b", a=1)  # [1, 256]
    xkv_g = xkv.rearrange("(g j p) c -> g p j c", j=8, p=128)   # [4,128,8,256]
    m_g = m_dram.rearrange("(j p) c -> p j c", j=2)
    wvo_g = wvo_dram.rearrange("(j p) c -> p j c", j=2)
    out_t = out.rearrange("(t p) c -> t p c", p=128)            # [16,128,256]

    with tile.TileContext(nc) as tc:
        with (
            tc.tile_pool(name="const", bufs=1) as cpool,
            tc.tile_pool(name="pt", bufs=6) as pt_pool,
            tc.tile_pool(name="sacc", bufs=2) as sacc_pool,
            tc.tile_pool(name="ovec", bufs=2) as ovec_pool,
            tc.tile_pool(name="fout", bufs=2) as fout_pool,
            tc.tile_pool(name="psmm", bufs=1, space="PSUM") as psmm,
            tc.tile_pool(name="psacc", bufs=1, space="PSUM") as psacc,
        ):
            # ---- constants (no DMA deps) ----
            warm = cpool.tile([128, 128], F32R, tag="warm", name="warm")
            nc.vector.memset(warm[:].bitcast(mybir.dt.uint32), 0x3F800000)
            ident = cpool.tile([128, 128], F32, tag="ident", name="ident")
            make_identity(nc, ident[:])
            ident_r = cpool.tile([128, 128], F32R, tag="identr", name="identr")
            nc.vector.tensor_copy(ident_r[:], ident[:])
            ones128 = cpool.tile([128, 128], F32R, tag="ones128",
                                 name="ones128")
            nc.vector.memset(ones128[:].bitcast(mybir.dt.uint32), 0x3F800000)


            # ---- PE warmup: dummy matmuls during the initial DMA window so
            # the tensor engine p-state ramp (full clock only after ~3us of
            # continuous busy) completes before real work arrives. Writes
            # rotate over disjoint PSUM slices: a write-after-write chain
            # would make every matmul wait on the previous one, and the cost
            # model restarts the ramp on every just-in-time wait. ----
            wps = psacc.tile([128, 512], F32, tag="accd", name="accd",
                             bufs=1)
            wi = [0]

            def warmup(n):
                for _ in range(n):
                    s = (wi[0] % 4) * 128
                    nc.tensor.matmul(wps[:, s:s + 128], warm[:], warm[:],
                                     start=True, stop=True)
                    wi[0] += 1

            warmup(14)

            # ---- input tiles + DMA order (earliest consumer first) ----
            xt = [cpool.tile([128, 8 * D], F32R, tag=f"xin{g}", name=f"xin{g}")
                  for g in range(4)]
            m_sb = cpool.tile([128, 2 * D], F32R, tag="m", name="m")
            wvo_sb = cpool.tile([128, 2 * D], F32R, tag="wvo", name="wvo")
            dpos = cpool.tile([128, 32], F32, tag="dpos", name="dpos")
            bo_sb = cpool.tile([1, D], F32, tag="bor", name="bor")

            xt0j = xt[0].rearrange("p (j c) -> p j c", j=8)
            nc.sync.dma_start(xt0j[:, 0:4], _r(xkv_g[0][:, 0:4]))
            nc.sync.dma_start(xt0j[:, 4:8], _r(xkv_g[0][:, 4:8]))
            nc.sync.dma_start(
                m_sb.rearrange("p (j c) -> p j c", j=2), _r(m_g[:]))
            nc.sync.dma_start(dpos[:], dpos_dram)
            nc.sync.dma_start(bo_sb[:], bo_row[:])
            for g in (1, 2, 3):
                nc.sync.dma_start(
                    xt[g].rearrange("p (j c) -> p j c", j=8), _r(xkv_g[g]))
            nc.sync.dma_start(
                wvo_sb.rearrange("p (j c) -> p j c", j=2), _r(wvo_g[:]))

            # ---- persistent activations ----
            xkvT = [cpool.tile([128, S], F32R, tag=f"xkvT{c}", name=f"xkvT{c}")
                    for c in range(2)]
            G = [cpool.tile([128, SQ], F32R, tag=f"G{c}", name=f"G{c}")
                 for c in range(2)]
            # bo as a rounded-f32r row: added inside the final projection via
            # a rank-1 ones-row matmul, so the eviction is a plain copy
            bo_r = cpool.tile([1, D], F32R, tag="bor2", name="bor2")
            nc.vector.tensor_copy(bo_r[:], bo_sb[:])

            ev = [0]

            def evict(dst, src):
                if ev[0] % 2 == 0:
                    nc.vector.tensor_copy(dst, src)
                else:
                    nc.scalar.copy(dst, src)
                ev[0] += 1

            def trans_grp(g, half, c):
                # x^T d-chunk c for rows [g*1024 + half*512, +512)
                tp = psmm.tile([128, 512], F32, tag="sc", name="sc", bufs=3)
                for j in range(4):
                    jj = half * 4 + j
                    nc.tensor.transpose(
                        _r(tp[:, j * 128:(j + 1) * 128]),
                        xt[g][:, jj * D + c * 128: jj * D + (c + 1) * 128],
                        ident_r[:])
                col0 = (g * 8 + half * 4) * 128
                evict(xkvT[c][:, col0:col0 + 512], tp[:])

            def qmt_grp(blk, c2):
                # G[c2][:, 512-query block] = (M^T x_q^T) e-chunk c2
                qsl = slice(blk * 512, (blk + 1) * 512)
                pp = psmm.tile([128, 512], F32, tag="sc", name="sc", bufs=3)
                for j in range(2):
                    nc.tensor.matmul(
                        pp[:],
                        m_sb[:, j * D + c2 * 128: j * D + (c2 + 1) * 128],
                        xkvT[j][:, qsl],
                        start=(j == 0), stop=(j == 1))
                evict(G[c2][:, qsl], pp[:])

            def ones_mm(ctx):
                # accd = column sums of P^T, from the two half-chains
                w = ctx["w"]
                nc.tensor.matmul(ctx["accd"][:, 0:w], ones128[:],
                                 ctx["sE"][:, 0:w], start=True, stop=False)
                nc.tensor.matmul(ctx["accd"][:, 0:w], ones128[:],
                                 ctx["sO"][:, 0:w], start=False, stop=True)

            def qscale(ctx, t4):
                # per-query-quarter 1/denom and Z^T scaling (all DVE; the
                # hardware Pool engine cannot read PSUM)
                if "rec" not in ctx:
                    ctx["rec"] = ovec_pool.tile([128, 512], F32, tag="rec",
                                                name="rec")
                    ctx["o"] = [ovec_pool.tile([128, 512], F32R, tag=f"o{e}",
                                               name=f"o{e}") for e in range(2)]
                tsl = slice(t4 * 128, (t4 + 1) * 128)
                nc.vector.reciprocal(ctx["rec"][:, tsl],
                                     ctx["accd"][:, tsl])
                for e in range(2):
                    nc.vector.tensor_mul(ctx["o"][e][:, tsl],
                                         ctx["acc"][e][:, tsl],
                                         ctx["rec"][:, tsl])

            def fp_t4(ctx, t4):
                # projection of one 128-query tile; bo enters as a rank-1
                # accumulating matmul so the eviction is a plain ACT copy
                tsl = slice(t4 * 128, (t4 + 1) * 128)
                fpt = psmm.tile([128, 512], F32, tag="sc", name="sc", bufs=3)
                fp = fpt[:, 0:D]
                for e in range(2):
                    nc.tensor.matmul(
                        fp, ctx["o"][e][:, tsl],
                        wvo_sb[:, e * D:(e + 1) * D],
                        start=(e == 0), stop=False)
                nc.tensor.matmul(fp, ones128[0:1, :], bo_r[:],
                                 start=False, stop=True)
                fo = fout_pool.tile([128, D], F32, tag="fout", name="fout",
                                    bufs=4)
                nc.scalar.copy(fo[:], fp)
                nc.sync.dma_start(out_t[ctx["qoff"] // 128 + t4], fo[:])

            # ---- prologue: first half of g0's x^T + G block 0 — just enough
            # to start the qb0 score loop; everything else streams in via
            # per-slot extras below, paced to DMA arrival.
            trans_grp(0, 0, 0)
            trans_grp(0, 0, 1)
            trans_grp(0, 1, 0)
            trans_grp(0, 1, 1)
            warmup(4)  # cover the eviction latency of the transposes
            qmt_grp(0, 0)
            qmt_grp(0, 1)
            qmt_grp(1, 0)
            qmt_grp(1, 1)
            warmup(3)  # cover the eviction latency of the G blocks

            # One PSUM-group of prologue work per scheduled slot: g1..g3
            # transposes feed qb0's later key tiles; G blocks 2-3 feed qb2/3.
            extras = {}

            def add_extra(qb, st, th):
                extras.setdefault((qb, st), []).append(th)

            slots = [3, 4, 6, 7, 10, 11, 14, 15, 18, 19, 22, 23]
            idx = 0
            for g in (1, 2, 3):
                for half in range(2):
                    for c in range(2):
                        add_extra(0, slots[idx],
                                  lambda g=g, half=half, c=c:
                                  trans_grp(g, half, c))
                        idx += 1
            slot = 7
            for blk in (2, 3):
                for c2 in range(2):
                    add_extra(1, slot,
                              lambda blk=blk, c2=c2: qmt_grp(blk, c2))
                    slot += 2

            blocks = [(0, 512), (512, 512), (1024, 512), (1536, 512)]
            ctxs = []
            for bi, (qoff, w) in enumerate(blocks):
                ls = bi == len(blocks) - 1
                qsl = slice(qoff, qoff + w)
                acc = [psacc.tile([128, 512], F32, tag=f"acc{e}",
                                  name=f"acc{e}", bufs=2) for e in range(2)]
                accd = psacc.tile([128, 512], F32, tag="accd", name="accd",
                                  bufs=1)
                ctx = {"qoff": qoff, "w": w, "nt": w // 128, "acc": acc,
                       "accd": accd}
                ctxs.append(ctx)
                prev = ctxs[bi - 1] if bi >= 1 else None

                pts = {}
                chains = {0: None, 1: None}

                def chain_step(k, w=w):
                    # two interleaved denominator chains: even key tiles
                    # accumulate on DVE, odd ones on Pool (SBUF-only engine)
                    if k < 2:
                        return
                    par = k % 2
                    eng = nc.vector if par == 0 else nc.gpsimd
                    t = sacc_pool.tile([128, 512], F32R, tag="sacc",
                                       name="sacc", bufs=4)
                    if k < 4:
                        eng.tensor_add(t[:, 0:w], pts[k - 2][:, 0:w],
                                       pts[k][:, 0:w])
                    else:
                        eng.tensor_add(t[:, 0:w], chains[par][:, 0:w],
                                       pts[k][:, 0:w])
                    chains[par] = t

                def pv_mm(k, acc=acc, w=w):
                    g, jj = k // 8, k % 8
                    for e in range(2):
                        nc.tensor.matmul(
                            acc[e][:, 0:w],
                            xt[g][:, jj * D + e * 128: jj * D + (e + 1) * 128],
                            pts[k][:, 0:w], start=(k == 0), stop=(k == 31))

                def boundary(st):
                    # previous block's denominator/scale/projection, spread
                    # so every op lands >=1 slot before its consumer
                    if st == 2:
                        ones_mm(prev)
                        qscale(prev, 0)
                        qscale(prev, 1)
                    elif st == 3:
                        for t4 in range(2, prev["nt"]):
                            qscale(prev, t4)
                        fp_t4(prev, 0)
                    elif st == 4:
                        fp_t4(prev, 1)
                    elif st in (5, 6) and prev["nt"] > 2:
                        fp_t4(prev, st - 3)

                # scores/exp run three slots ahead of PV + denominator chain
                # so the PE never waits on the activation engine's exp
                # latency, even in slots carrying boundary extras.
                for st in range(32):
                    for th in extras.get((bi, st), ()):
                        th()
                    # scores^T for key tile st (contract over e, 2 chunks)
                    ssl = slice(st * 128, (st + 1) * 128)
                    sp = psmm.tile([128, 512], F32, tag="sc", name="sc",
                                   bufs=3)
                    nc.tensor.matmul(sp[:, 0:w], xkvT[0][:, ssl],
                                     G[0][:, qsl], start=True, stop=False)
                    nc.tensor.matmul(sp[:, 0:w], xkvT[1][:, ssl],
                                     G[1][:, qsl], start=False, stop=True)
                    pt = pt_pool.tile([128, 512], F32R, tag="pt", name="pt",
                                      bufs=8)
                    nc.scalar.activation(pt[:, 0:w], sp[:, 0:w], EXP,
                                         scale=SCALE,
                                         bias=dpos[:, st:st + 1])
                    pts[st] = pt
                    if st >= 3:
                        pv_mm(st - 3)
                        chain_step(st - 3)
                    if prev is not None:
                        boundary(st)
                # drain the +3 lag; for the last block the denominator is
                # finished on the PE (4-piece accumulation over the two
                # half-chains and the last two exps) so its tail does not
                # wait for the final chain adds.
                pv_mm(29)
                chain_step(29)
                pv_mm(30)
                if not ls:
                    chain_step(30)
                    pv_mm(31)
                    chain_step(31)
                    ctx["sE"] = chains[0]
                    ctx["sO"] = chains[1]
                else:
                    pv_mm(31)
                    nc.tensor.matmul(accd[:, 0:w], ones128[:],
                                     chains[0][:, 0:w], start=True,
                                     stop=False)
                    nc.tensor.matmul(accd[:, 0:w], ones128[:],
                                     chains[1][:, 0:w], start=False,
                                     stop=False)
                    nc.tensor.matmul(accd[:, 0:w], ones128[:],
                                     pts[30][:, 0:w], start=False, stop=False)
                    nc.tensor.matmul(accd[:, 0:w], ones128[:],
                                     pts[31][:, 0:w], start=False, stop=True)

            # ---- final block tail ----
            last = ctxs[-1]
            for t4 in range(last["nt"]):
                qscale(last, t4)
                fp_t4(last, t4)

    nc.compile()
    return nc


_NC = None


def _get_nc():
    global _NC
    if _NC is None:
        _NC = _build()
    return _NC


def _make_in_maps(x, Wq, bq, Wk, bk, Wv, bv, Wo, bo):
    """Host-side prep: weight folds + per-core rotation.

    M = Wq Wk^T and Wvo = Wv Wo are exact weight-weight folds; bv folds into
    bo (attention rows sum to 1); the only bias term that is not
    softmax-invariant is the per-key d = x_k (Wk bq), shipped pre-tiled and
    pre-scaled as dpos[128, 32]."""
    M = (Wq @ Wk.T).astype(np.float32)
    Wvo = (Wv @ Wo).astype(np.float32)
    bo_eff = (bv @ Wo + bo).astype(np.float32)
    u = (Wk @ bq).astype(np.float32)
    in_maps = []
    for c in range(NCORES):
        b, h = divmod(c, 2)
        xb = x[b] if h == 0 else np.ascontiguousarray(
            np.concatenate([x[b, SQ:], x[b, :SQ]]))
        d = (xb @ u) * np.float32(SCALE)
        dpos = np.ascontiguousarray(d.reshape(32, 128).T).astype(np.float32)
        in_maps.append({
            "xkv": xb, "mqk": M, "wvo": Wvo, "dpos": dpos, "bo": bo_eff,
        })
    return in_maps


class _Runner:
    """Cached jitted SPMD executor (run_bass_kernel_spmd rebuilds its jax
    closure every call, forcing a retrace; this traces once)."""

    def __init__(self, nc):
        import jax
        from jax.sharding import Mesh, PartitionSpec
        from jax.experimental.shard_map import shard_map
        from concourse import bass2jax, mybir as mb

        bass2jax.install_neuronx_cc_hook()
        self.jax = jax
        if not any("axon" in str(getattr(d, "platform", "")).lower()
                   or str(d).startswith("NC_")
                   for d in jax.devices()):
            import jax._src.xla_bridge as xb
            jax.config.update("jax_platforms", None)
            xb._clear_backends()
            if hasattr(xb.get_backend, "cache_clear"):
                xb.get_backend.cache_clear()
            if not any("axon" in str(getattr(d, "platform", "")).lower()
                       or str(d).startswith("NC_")
                       for d in jax.devices()):
                jax.config.update("jax_platforms", "axon")
                xb._clear_backends()
                if hasattr(xb.get_backend, "cache_clear"):
                    xb.get_backend.cache_clear()
        partition_name = (nc.partition_id_tensor.name
                          if nc.partition_id_tensor else None)
        in_names, out_names, out_avals = [], [], []
        for alloc in nc.m.functions[0].allocations:
            if not isinstance(alloc, mb.MemoryLocationSet):
                continue
            name = alloc.memorylocations[0].name
            if alloc.kind == "ExternalInput":
                if name != partition_name:
                    in_names.append(name)
            elif alloc.kind == "ExternalOutput":
                out_names.append(name)
                out_avals.append(jax.core.ShapedArray(
                    tuple(alloc.tensor_shape), mb.dt.np(alloc.dtype)))
        self.in_names, self.out_names, self.out_avals = \
            in_names, out_names, out_avals
        n_params, n_outs = len(in_names), len(out_names)
        bind_in_names = in_names + out_names + (
            [partition_name] if partition_name else [])

        def _body(*args):
            operands = list(args)
            if partition_name is not None:
                operands.append(bass2jax.partition_id_tensor())
            outs = bass2jax._bass_exec_p.bind(
                *operands,
                out_avals=tuple(out_avals),
                in_names=tuple(bind_in_names),
                out_names=tuple(out_names),
                lowering_input_output_aliases=(),
                sim_require_finite=True,
                sim_require_nnan=True,
                nc=nc,
            )
            return tuple(outs)

        devices = jax.devices()[:NCORES]
        mesh = Mesh(np.asarray(devices), ("core",))
        spec = (PartitionSpec("core"),) * (n_params + n_outs)
        self.fn = jax.jit(
            shard_map(_body, mesh=mesh, in_specs=spec,
                      out_specs=(PartitionSpec("core"),) * n_outs,
                      check_rep=False),
            donate_argnums=tuple(range(n_params, n_params + n_outs)),
            keep_unused=True,
        )

    def run(self, in_maps):
        concat_in = [
            np.concatenate([np.asarray(m[n]) for m in in_maps], axis=0)
            for n in self.in_names
        ]
        concat_zeros = [
            np.zeros((NCORES * a.shape[0], *a.shape[1:]), a.dtype)
            for a in self.out_avals
        ]
        outs = self.fn(*concat_in, *concat_zeros)
        return [
            {n: np.asarray(outs[i]).reshape(NCORES, *self.out_avals[i].shape)[c]
             for i, n in enumerate(self.out_names)}
            for c in range(NCORES)
        ]


_RUNNER = None


def _get_runner():
    global _RUNNER
    if _RUNNER is None:
        _RUNNER = _Runner(_get_nc())
    return _RUNNER


def kernel(**inputs):
    x = np.ascontiguousarray(np.asarray(inputs["x"], dtype=np.float32))
    Wq = np.ascontiguousarray(np.asarray(inputs["Wq"], dtype=np.float32))
    Wk = np.ascontiguousarray(np.asarray(inputs["Wk"], dtype=np.float32))
    Wv = np.ascontiguousarray(np.asarray(inputs["Wv"], dtype=np.float32))
    Wo = np.ascontiguousarray(np.asarray(inputs["Wo"], dtype=np.float32))
    bq = np.ascontiguousarray(np.asarray(inputs["bq"], dtype=np.float32))
    bk = np.ascontiguousarray(np.asarray(inputs["bk"], dtype=np.float32))
    bv = np.ascontiguousarray(np.asarray(inputs["bv"], dtype=np.float32))
    bo = np.ascontiguousarray(np.asarray(inputs["bo"], dtype=np.float32))

    try:
        runner = _get_runner()
    except Exception:
        runner = None
    in_maps = _make_in_maps(x, Wq, bq, Wk, bk, Wv, bv, Wo, bo)
    results = None
    if runner is not None:
        try:
            results = runner.run(in_maps)
        except Exception:
            results = None
    if results is None:
        results = run_bass_kernel_spmd(
            _get_nc(), in_maps, core_ids=list(range(NCORES))).results
    outp = np.empty((B, S, D), dtype=np.float32)
    for c in range(NCORES):
        b, h = divmod(c, 2)
        outp[b, h * SQ:(h + 1) * SQ] = results[c]["out"]
    return outp


# revision 52
# speedup vs baseline: 1.7461x; 1.0103x over previous
"""Trainium2 Bass kernel: single-head attention module (dense transformer).

Computes, for x [4, 4096, 256] (f32) and per-projection weights/biases:
    q = x @ Wq + bq;  k = x @ Wk + bk;  v = x @ Wv + bv
    out = softmax((q k^T) / sqrt(256)) @ v @ Wo + bo

Sharding over 8 NeuronCores: core c handles batch c//2, query half c%2.
The host rotates each core's batch so its queries are always rows 0..2047.

Algebraic restructure vs the straightforward kernel (weight-weight products
are precomputed host-side; they are 256^3 and exact):
  - scores = x_q (Wq Wk^T) x_k^T + per-key bias d, with M = Wq Wk^T and
    d = x_k (Wk bq) (per-query terms and constants are softmax-invariant).
    This removes the K projection entirely; x^T serves as the key operand.
  - out = (P x_k) (Wv Wo) / denom + (bv Wo + bo): reassociating P V Wo as
    (P x) Wvo removes the V projection; natural-layout x tiles are the
    stationary operand of the PV matmul, Wvo = Wv Wo folds the two output
    projections into one.
Device work per core: 64 transposes (x^T), G = M^T x_q^T (8k cycles),
scores (131k), PV (131k), softmax denominator via an eager DVE add-chain +
one ones-matmul per 512-query block (2k), final projection (8k).

Emission is software-pipelined: scores/exp run one step ahead of PV; the
g1..g3 transposes and the G blocks for later query-blocks are spread into
early slots of the first two query-block loops; each block's denominator /
reciprocal / final projection is interleaved into the next block's first
four slots so the PE never drains.
"""

import numpy as np

import concourse.bass as bass  # noqa: F401
import concourse.tile as tile
from concourse import bacc, mybir
from concourse.bass_utils import run_bass_kernel_spmd
from concourse.masks import make_identity

B, S, D = 4, 4096, 256
SQ = S // 2  # queries per core
NCORES = 8
F32 = mybir.dt.float32
F32R = mybir.dt.float32r
BF16 = mybir.dt.bfloat16
SCALE = 1.0 / 16.0  # 1/sqrt(D)
EXP = mybir.ActivationFunctionType.Exp


def _r(ap):
    """View an fp32 AP as float32r: full-rate fp32 matmul on the PE."""
    return ap.bitcast(F32R)


def _build():
    nc = bacc.Bacc("TRN2", target_bir_lowering=False, debug=False,
                   num_devices=NCORES)

    xkv = nc.dram_tensor("xkv", [S, D], F32, kind="ExternalInput").ap()
    m_dram = nc.dram_tensor("mqk", [D, D], F32, kind="ExternalInput").ap()
    wvo_dram = nc.dram_tensor("wvo", [D, D], F32, kind="ExternalInput").ap()
    dpos_dram = nc.dram_tensor("dpos", [128, 32], F32,
                               kind="ExternalInput").ap()
    bo_dram = nc.dram_tensor("bo", [D], F32, kind="ExternalInput").ap()
    out = nc.dram_tensor("out", [SQ, D], F32, kind="ExternalOutput").ap()

    bo_row = bo_dram.rearrange("(a b) -> a b", a=1)  # [1, 256]
    xkv_g = xkv.rearrange("(g j p) c -> g p j c", j=8, p=128)   # [4,128,8,256]
    m_g = m_dram.rearrange("(j p) c -> p j c", j=2)
    wvo_g = wvo_dram.rearrange("(j p) c -> p j c", j=2)
    out_t = out.rearrange("(t p) c -> t p c", p=128)            # [16,128,256]

    with tile.TileContext(nc) as tc:
        with (
            tc.tile_pool(name="const", bufs=1) as cpool,
            tc.tile_pool(name="pt", bufs=6) as pt_pool,
            tc.tile_pool(name="sacc", bufs=2) as sacc_pool,
            tc.tile_pool(name="ovec", bufs=2) as ovec_pool,
            tc.tile_pool(name="fout", bufs=2) as fout_pool,
            tc.tile_pool(name="psmm", bufs=1, space="PSUM") as psmm,
            tc.tile_pool(name="psacc", bufs=1, space="PSUM") as psacc,
        ):
            # ---- constants (no DMA deps) ----
            warm = cpool.tile([128, 128], F32R, tag="warm", name="warm")
            nc.vector.memset(warm[:].bitcast(mybir.dt.uint32), 0x3F800000)
            ident = cpool.tile([128, 128], F32, tag="ident", name="ident")
            make_identity(nc, ident[:])
            ident_b = cpool.tile([128, 128], BF16, tag="identb", name="identb")
            nc.vector.tensor_copy(ident_b[:], ident[:])
            ones128 = cpool.tile([128, 128], BF16, tag="ones128",
                                 name="ones128")
            nc.vector.memset(ones128[:].bitcast(mybir.dt.uint16), 0x3F80)
            ones_r = cpool.tile([1, 128], F32R, tag="onesr", name="onesr")
            nc.vector.memset(ones_r[:].bitcast(mybir.dt.uint32), 0x3F800000)


            # ---- PE warmup: dummy matmuls during the initial DMA window so
            # the tensor engine p-state ramp (full clock only after ~3us of
            # continuous busy) completes before real work arrives. Writes
            # rotate over disjoint PSUM slices: a write-after-write chain
            # would make every matmul wait on the previous one, and the cost
            # model restarts the ramp on every just-in-time wait. ----
            wps = psacc.tile([128, 512], F32, tag="accd", name="accd",
                             bufs=1)
            wi = [0]

            def warmup(n):
                for _ in range(n):
                    s = (wi[0] % 4) * 128
                    nc.tensor.matmul(wps[:, s:s + 128], warm[:], warm[:],
                                     start=True, stop=True)
                    wi[0] += 1

            warmup(14)

            # ---- input tiles + DMA order (earliest consumer first) ----
            # x and M load as bf16 via casting SWDGE DMAs on the Pool engine
            # (half the bytes; bf16 transposes run 1 cycle/row on the PE)
            xt = [cpool.tile([128, 8 * D], BF16, tag=f"xin{g}", name=f"xin{g}")
                  for g in range(4)]
            m_sb = cpool.tile([128, 2 * D], BF16, tag="m", name="m")
            wvo_sb = cpool.tile([128, 2 * D], F32R, tag="wvo", name="wvo")
            dpos = cpool.tile([128, 32], F32, tag="dpos", name="dpos")
            bo_sb = cpool.tile([1, D], F32, tag="bor", name="bor")

            xt0j = xt[0].rearrange("p (j c) -> p j c", j=8)
            nc.gpsimd.dma_start(xt0j[:, 0:4], xkv_g[0][:, 0:4])
            nc.gpsimd.dma_start(xt0j[:, 4:8], xkv_g[0][:, 4:8])
            nc.gpsimd.dma_start(
                m_sb.rearrange("p (j c) -> p j c", j=2), m_g[:])
            nc.sync.dma_start(dpos[:], dpos_dram)
            nc.sync.dma_start(bo_sb[:], bo_row[:])
            for g in (1, 2, 3):
                nc.gpsimd.dma_start(
                    xt[g].rearrange("p (j c) -> p j c", j=8), xkv_g[g])
            nc.sync.dma_start(
                wvo_sb.rearrange("p (j c) -> p j c", j=2), _r(wvo_g[:]))

            # ---- persistent activations ----
            xkvT = [cpool.tile([128, S], BF16, tag=f"xkvT{c}", name=f"xkvT{c}")
                    for c in range(2)]
            G = [cpool.tile([128, SQ], BF16, tag=f"G{c}", name=f"G{c}")
                 for c in range(2)]
            # bo as a rounded-f32r row: added inside the final projection via
            # a rank-1 ones-row matmul, so the eviction is a plain copy
            bo_r = cpool.tile([1, D], F32R, tag="bor2", name="bor2")
            nc.vector.tensor_copy(bo_r[:], bo_sb[:])

            ev = [0]

            def evict(dst, src):
                if ev[0] % 2 == 0:
                    nc.vector.tensor_copy(dst, src)
                else:
                    nc.scalar.copy(dst, src)
                ev[0] += 1

            def trans_grp(g, half, c):
                # x^T d-chunk c for rows [g*1024 + half*512, +512); bf16
                # transposes land in a bf16 view of the PSUM bank
                tp = psmm.tile([128, 512], F32, tag="sc", name="sc", bufs=3)
                tpb = tp[:].bitcast(BF16)
                for j in range(4):
                    jj = half * 4 + j
                    nc.tensor.transpose(
                        tpb[:, j * 128:(j + 1) * 128],
                        xt[g][:, jj * D + c * 128: jj * D + (c + 1) * 128],
                        ident_b[:])
                col0 = (g * 8 + half * 4) * 128
                evict(xkvT[c][:, col0:col0 + 512], tpb[:, 0:512])

            def qmt_grp(blk, c2):
                # G[c2][:, 512-query block] = (M^T x_q^T) e-chunk c2
                qsl = slice(blk * 512, (blk + 1) * 512)
                pp = psmm.tile([128, 512], F32, tag="sc", name="sc", bufs=3)
                for j in range(2):
                    nc.tensor.matmul(
                        pp[:],
                        m_sb[:, j * D + c2 * 128: j * D + (c2 + 1) * 128],
                        xkvT[j][:, qsl],
                        start=(j == 0), stop=(j == 1))
                evict(G[c2][:, qsl], pp[:])

            def ones_mm(ctx):
                # accd = column sums of P^T, from the two half-chains
                w = ctx["w"]
                nc.tensor.matmul(ctx["accd"][:, 0:w], ones128[:],
                                 ctx["sE"][:, 0:w], start=True, stop=False)
                nc.tensor.matmul(ctx["accd"][:, 0:w], ones128[:],
                                 ctx["sO"][:, 0:w], start=False, stop=True)

            def qscale(ctx, t4):
                # per-query-quarter 1/denom and Z^T scaling (all DVE; the
                # hardware Pool engine cannot read PSUM)
                if "rec" not in ctx:
                    ctx["rec"] = ovec_pool.tile([128, 512], F32, tag="rec",
                                                name="rec")
                    ctx["o"] = [ovec_pool.tile([128, 512], F32R, tag=f"o{e}",
                                               name=f"o{e}") for e in range(2)]
                tsl = slice(t4 * 128, (t4 + 1) * 128)
                nc.vector.reciprocal(ctx["rec"][:, tsl],
                                     ctx["accd"][:, tsl])
                for e in range(2):
                    nc.vector.tensor_mul(ctx["o"][e][:, tsl],
                                         ctx["acc"][e][:, tsl],
                                         ctx["rec"][:, tsl])

            def fp_t4(ctx, t4):
                # projection of one 128-query tile; bo enters as a rank-1
                # accumulating matmul so the eviction is a plain ACT copy
                tsl = slice(t4 * 128, (t4 + 1) * 128)
                fpt = psmm.tile([128, 512], F32, tag="sc", name="sc", bufs=3)
                fp = fpt[:, 0:D]
                for e in range(2):
                    nc.tensor.matmul(
                        fp, ctx["o"][e][:, tsl],
                        wvo_sb[:, e * D:(e + 1) * D],
                        start=(e == 0), stop=False)
                nc.tensor.matmul(fp, ones_r[:], bo_r[:],
                                 start=False, stop=True)
                fo = fout_pool.tile([128, D], F32, tag="fout", name="fout",
                                    bufs=4)
                nc.scalar.copy(fo[:], fp)
                nc.sync.dma_start(out_t[ctx["qoff"] // 128 + t4], fo[:])

            # ---- prologue: first half of g0's x^T + G block 0 — just enough
            # to start the qb0 score loop; everything else streams in via
            # per-slot extras below, paced to DMA arrival.
            trans_grp(0, 0, 0)
            trans_grp(0, 0, 1)
            trans_grp(0, 1, 0)
            trans_grp(0, 1, 1)
            warmup(4)  # cover the eviction latency of the transposes
            qmt_grp(0, 0)
            qmt_grp(0, 1)
            qmt_grp(1, 0)
            qmt_grp(1, 1)
            warmup(3)  # cover the eviction latency of the G blocks

            # One PSUM-group of prologue work per scheduled slot: g1..g3
            # transposes feed qb0's later key tiles; G blocks 2-3 feed qb2/3.
            extras = {}

            def add_extra(qb, st, th):
                extras.setdefault((qb, st), []).append(th)

            slots = [3, 4, 6, 7, 10, 11, 14, 15, 18, 19, 22, 23]
            idx = 0
            for g in (1, 2, 3):
                for half in range(2):
                    for c in range(2):
                        add_extra(0, slots[idx],
                                  lambda g=g, half=half, c=c:
                                  trans_grp(g, half, c))
                        idx += 1
            slot = 7
            for blk in (2, 3):
                for c2 in range(2):
                    add_extra(1, slot,
                              lambda blk=blk, c2=c2: qmt_grp(blk, c2))
                    slot += 2

            blocks = [(0, 512), (512, 512), (1024, 512), (1536, 512)]
            ctxs = []
            for bi, (qoff, w) in enumerate(blocks):
                ls = bi == len(blocks) - 1
                qsl = slice(qoff, qoff + w)
                acc = [psacc.tile([128, 512], F32, tag=f"acc{e}",
                                  name=f"acc{e}", bufs=2) for e in range(2)]
                accd = psacc.tile([128, 512], F32, tag="accd", name="accd",
                                  bufs=1)
                ctx = {"qoff": qoff, "w": w, "nt": w // 128, "acc": acc,
                       "accd": accd}
                ctxs.append(ctx)
                prev = ctxs[bi - 1] if bi >= 1 else None

                pts = {}
                chains = {0: None, 1: None}

                def chain_step(k, w=w):
                    # two interleaved denominator chains: even key tiles
                    # accumulate on DVE, odd ones on Pool (SBUF-only engine)
                    if k < 2:
                        return
                    par = k % 2
                    eng = nc.vector if par == 0 else nc.gpsimd
                    t = sacc_pool.tile([128, 512], BF16, tag="sacc",
                                       name="sacc", bufs=4)
                    if k < 4:
                        eng.tensor_add(t[:, 0:w], pts[k - 2][:, 0:w],
                                       pts[k][:, 0:w])
                    else:
                        eng.tensor_add(t[:, 0:w], chains[par][:, 0:w],
                                       pts[k][:, 0:w])
                    chains[par] = t

                def pv_mm(k, acc=acc, w=w):
                    g, jj = k // 8, k % 8
                    for e in range(2):
                        nc.tensor.matmul(
                            acc[e][:, 0:w],
                            xt[g][:, jj * D + e * 128: jj * D + (e + 1) * 128],
                            pts[k][:, 0:w], start=(k == 0), stop=(k == 31))

                def boundary(st):
                    # previous block's denominator/scale/projection, spread
                    # so every op lands >=1 slot before its consumer
                    if st == 2:
                        ones_mm(prev)
                        qscale(prev, 0)
                        qscale(prev, 1)
                    elif st == 3:
                        for t4 in range(2, prev["nt"]):
                            qscale(prev, t4)
                        fp_t4(prev, 0)
                    elif st == 4:
                        fp_t4(prev, 1)
                    elif st in (5, 6) and prev["nt"] > 2:
                        fp_t4(prev, st - 3)

                # scores/exp run three slots ahead of PV + denominator chain
                # so the PE never waits on the activation engine's exp
                # latency, even in slots carrying boundary extras.
                for st in range(32):
                    for th in extras.get((bi, st), ()):
                        th()
                    # scores^T for key tile st (contract over e, 2 chunks)
                    ssl = slice(st * 128, (st + 1) * 128)
                    sp = psmm.tile([128, 512], F32, tag="sc", name="sc",
                                   bufs=3)
                    nc.tensor.matmul(sp[:, 0:w], xkvT[0][:, ssl],
                                     G[0][:, qsl], start=True, stop=False)
                    nc.tensor.matmul(sp[:, 0:w], xkvT[1][:, ssl],
                                     G[1][:, qsl], start=False, stop=True)
                    pt = pt_pool.tile([128, 512], BF16, tag="pt", name="pt",
                                      bufs=8)
                    nc.scalar.activation(pt[:, 0:w], sp[:, 0:w], EXP,
                                         scale=SCALE,
                                         bias=dpos[:, st:st + 1])
                    pts[st] = pt
                    if st >= 3:
                        pv_mm(st - 3)
                        chain_step(st - 3)
                    if prev is not None:
                        boundary(st)
                # drain the +3 lag; for the last block the denominator is
                # finished on the PE (4-piece accumulation over the two
                # half-chains and the last two exps) so its tail does not
                # wait for the final chain adds.
                pv_mm(29)
                chain_step(29)
                pv_mm(30)
                if not ls:
                    chain_step(30)
                    pv_mm(31)
                    chain_step(31)
                    ctx["sE"] = chains[0]
                    ctx["sO"] = chains[1]
                else:
                    pv_mm(31)
                    nc.tensor.matmul(accd[:, 0:w], ones128[:],
                                     chains[0][:, 0:w], start=True,
                                     stop=False)
                    nc.tensor.matmul(accd[:, 0:w], ones128[:],
                                     chains[1][:, 0:w], start=False,
                                     stop=False)
                    nc.tensor.matmul(accd[:, 0:w], ones128[:],
                                     pts[30][:, 0:w], start=False, stop=False)
                    nc.tensor.matmul(accd[:, 0:w], ones128[:],
                                     pts[31][:, 0:w], start=False, stop=True)

            # ---- final block tail ----
            last = ctxs[-1]
            for t4 in range(last["nt"]):
                qscale(last, t4)
                fp_t4(last, t4)

    nc.compile()
    return nc


_NC = None


def _get_nc():
    global _NC
    if _NC is None:
        _NC = _build()
    return _NC


def _make_in_maps(x, Wq, bq, Wk, bk, Wv, bv, Wo, bo):
    """Host-side prep: weight folds + per-core rotation.

    M = Wq Wk^T and Wvo = Wv Wo are exact weight-weight folds; bv folds into
    bo (attention rows sum to 1); the only bias term that is not
    softmax-invariant is the per-key d = x_k (Wk bq), shipped pre-tiled and
    pre-scaled as dpos[128, 32]."""
    M = (Wq @ Wk.T).astype(np.float32)
    Wvo = (Wv @ Wo).astype(np.float32)
    bo_eff = (bv @ Wo + bo).astype(np.float32)
    u = (Wk @ bq).astype(np.float32)
    in_maps = []
    for c in range(NCORES):
        b, h = divmod(c, 2)
        xb = x[b] if h == 0 else np.ascontiguousarray(
            np.concatenate([x[b, SQ:], x[b, :SQ]]))
        d = (xb @ u) * np.float32(SCALE)
        dpos = np.ascontiguousarray(d.reshape(32, 128).T).astype(np.float32)
        in_maps.append({
            "xkv": xb, "mqk": M, "wvo": Wvo, "dpos": dpos, "bo": bo_eff,
        })
    return in_maps


class _Runner:
    """Cached jitted SPMD executor (run_bass_kernel_spmd rebuilds its jax
    closure every call, forcing a retrace; this traces once)."""

    def __init__(self, nc):
        import jax
        from jax.sharding import Mesh, PartitionSpec
        from jax.experimental.shard_map import shard_map
        from concourse import bass2jax, mybir as mb

        bass2jax.install_neuronx_cc_hook()
        self.jax = jax
        if not any("axon" in str(getattr(d, "platform", "")).lower()
                   or str(d).startswith("NC_")
                   for d in jax.devices()):
            import jax._src.xla_bridge as xb
            jax.config.update("jax_platforms", None)
            xb._clear_backends()
            if hasattr(xb.get_backend, "cache_clear"):
                xb.get_backend.cache_clear()
            if not any("axon" in str(getattr(d, "platform", "")).lower()
                       or str(d).startswith("NC_")
                       for d in jax.devices()):
                jax.config.update("jax_platforms", "axon")
                xb._clear_backends()
                if hasattr(xb.get_backend, "cache_clear"):
                    xb.get_backend.cache_clear()
        partition_name = (nc.partition_id_tensor.name
                          if nc.partition_id_tensor else None)
        in_names, out_names, out_avals = [], [], []
        for alloc in nc.m.functions[0].allocations:
            if not isinstance(alloc, mb.MemoryLocationSet):
                continue
            name = alloc.memorylocations[0].name
            if alloc.kind == "ExternalInput":
                if name != partition_name:
                    in_names.append(name)
            elif alloc.kind == "ExternalOutput":
                out_names.append(name)
                out_avals.append(jax.core.ShapedArray(
                    tuple(alloc.tensor_shape), mb.dt.np(alloc.dtype)))
        self.in_names, self.out_names, self.out_avals = \
            in_names, out_names, out_avals
        n_params, n_outs = len(in_names), len(out_names)
        bind_in_names = in_names + out_names + (
            [partition_name] if partition_name else [])

        def _body(*args):
            operands = list(args)
            if partition_name is not None:
                operands.append(bass2jax.partition_id_tensor())
            outs = bass2jax._bass_exec_p.bind(
                *operands,
                out_avals=tuple(out_avals),
                in_names=tuple(bind_in_names),
                out_names=tuple(out_names),
                lowering_input_output_aliases=(),
                sim_require_finite=True,
                sim_require_nnan=True,
                nc=nc,
            )
            return tuple(outs)

        devices = jax.devices()[:NCORES]
        mesh = Mesh(np.asarray(devices), ("core",))
        spec = (PartitionSpec("core"),) * (n_params + n_outs)
        self.fn = jax.jit(
            shard_map(_body, mesh=mesh, in_specs=spec,
                      out_specs=(PartitionSpec("core"),) * n_outs,
                      check_rep=False),
            donate_argnums=tuple(range(n_params, n_params + n_outs)),
            keep_unused=True,
        )

    def run(self, in_maps):
        concat_in = [
            np.concatenate([np.asarray(m[n]) for m in in_maps], axis=0)
            for n in self.in_names
        ]
        concat_zeros = [
            np.zeros((NCORES * a.shape[0], *a.shape[1:]), a.dtype)
            for a in self.out_avals
        ]
        outs = self.fn(*concat_in, *concat_zeros)
        return [
            {n: np.asarray(outs[i]).reshape(NCORES, *self.out_avals[i].shape)[c]
             for i, n in enumerate(self.out_names)}
            for c in range(NCORES)
        ]


_RUNNER = None


def _get_runner():
    global _RUNNER
    if _RUNNER is None:
        _RUNNER = _Runner(_get_nc())
    return _RUNNER


def kernel(**inputs):
    x = np.ascontiguousarray(np.asarray(inputs["x"], dtype=np.float32))
    Wq = np.ascontiguousarray(np.asarray(inputs["Wq"], dtype=np.float32))
    Wk = np.ascontiguousarray(np.asarray(inputs["Wk"], dtype=np.float32))
    Wv = np.ascontiguousarray(np.asarray(inputs["Wv"], dtype=np.float32))
    Wo = np.ascontiguousarray(np.asarray(inputs["Wo"], dtype=np.float32))
    bq = np.ascontiguousarray(np.asarray(inputs["bq"], dtype=np.float32))
    bk = np.ascontiguousarray(np.asarray(inputs["bk"], dtype=np.float32))
    bv = np.ascontiguousarray(np.asarray(inputs["bv"], dtype=np.float32))
    bo = np.ascontiguousarray(np.asarray(inputs["bo"], dtype=np.float32))

    try:
        runner = _get_runner()
    except Exception:
        runner = None
    in_maps = _make_in_maps(x, Wq, bq, Wk, bk, Wv, bv, Wo, bo)
    results = None
    if runner is not None:
        try:
            results = runner.run(in_maps)
        except Exception:
            results = None
    if results is None:
        results = run_bass_kernel_spmd(
            _get_nc(), in_maps, core_ids=list(range(NCORES))).results
    outp = np.empty((B, S, D), dtype=np.float32)
    for c in range(NCORES):
        b, h = divmod(c, 2)
        outp[b, h * SQ:(h + 1) * SQ] = results[c]["out"]
    return outp
